# revision 1
# baseline (speedup 1.0000x reference)
"""Multi-head attention (B=4, N=2048, C=1024, H=16, D=64) on 8 trn2 cores.

Sharding: core c = (batch b = c//2, head-half g = c%2). Each core computes
attention for 8 heads of one batch plus the partial output projection over
its 512 channels; the host sums the two partials per batch and adds b_proj.

Device layout (all matmul operands bf16, fp32 PSUM accumulation):
  - host passes xT = x[b].T                       [1024, 2048]
  - QK^T:  qkT[m, n]  = wqk[:, m].T @ xT          (lhsT=wqk, rhs=xT)
  - V:     v[n, vc]   = xT[:, n].T @ wv           (lhsT=xT,  rhs=wv)
           stored interleaved as [V_h | ones] blocks of 65 per head
  - S^T:   s[m, n]    = kT[:, m].T @ qT[:, n]     (per head, contraction d=64)
  - E = exp(s / 8) via ScalarE (scores are O(1): no max subtraction needed)
  - PV:    u[dv, n]   = V1[m, dv].T @ E[m, n]     row 64 = softmax denom
  - norm:  attnT = u[0:64] * broadcast(1/u[64])   (PE K=1 broadcast matmul)
  - proj:  out[n, oc] = attnT[:, n].T @ wp        partial over 512 channels
"""

import numpy as np
import ml_dtypes

B, N, C = 4, 2048, 1024
H, D = 16, 64
HPC = 8            # heads per core
QKC = HPC * D      # 512 q (and k, v) channels per core
NCORES = 8

_nc_cache = None


def build_nc(split_waits=True, repeat=1):
    import concourse.bass as bass
    import concourse.mybir as mybir
    import concourse.tile as tile

    bf16 = mybir.dt.bfloat16
    f32 = mybir.dt.float32
    f32r = mybir.dt.float32r

    nc = bass.Bass()
    xT_d = nc.declare_dram_parameter("xT", [C, N], bf16, isOutput=False)
    wqk_d = nc.declare_dram_parameter("wqk", [C, 2 * QKC], bf16, isOutput=False)
    wv_d = nc.declare_dram_parameter("wv", [C, QKC], bf16, isOutput=False)
    wp_d = nc.declare_dram_parameter("wp", [QKC, C], bf16, isOutput=False)
    out_d = nc.declare_dram_parameter("out", [N, C], f32, isOutput=True)

    rdram = nc.dram_tensor("rscratch", [32, 512], f32)

    KT = C // 128          # 8 contraction tiles for qkv projection
    NT = N // 128          # 16 seq tiles
    NB = N // 512          # 4 seq blocks of 512
    QKT = 2 * QKC // 128   # 8 qk-channel tiles

    with tile.TileContext(nc) as tc:
        with (
            tc.tile_pool(name="big", bufs=1) as big,
            tc.tile_pool(name="work", bufs=18) as workp,
            tc.tile_pool(name="outp", bufs=4) as outp,
            tc.tile_pool(name="small", bufs=4) as smallp,
            tc.tile_pool(name="mm", bufs=2, space="PSUM") as mmp,
            tc.tile_pool(name="spsum", bufs=2, space="PSUM") as spsum,
            tc.tile_pool(name="pvpsum", bufs=2, space="PSUM") as pvpsum,
        ):
            # ---- load inputs ----
            xt = big.tile([128, KT, N], bf16, tag="xt")
            wqk = big.tile([128, KT, 2 * QKC], bf16, tag="wqk")
            wv = big.tile([128, KT, QKC], bf16, tag="wv")
            wp = big.tile([128, QKC // 128, C], bf16, tag="wp")
            xT_r = xT_d.rearrange("(t p) n -> p t n", p=128)
            wqk_r = wqk_d.rearrange("(t p) m -> p t m", p=128)
            wv_r = wv_d.rearrange("(t p) m -> p t m", p=128)
            wp_r = wp_d.rearrange("(t p) m -> p t m", p=128)
            # Each DMA instruction costs ~650ns of serialized issue on the
            # sync sequencer, so use few, large DMAs in consumption order.
            # wqk is host-reordered pair-major ([q|k] 256-col block per head
            # pair) so one DMA loads exactly what head pair 0 needs first.
            # leading chunks split in half so the first qk group's kt=0..3
            # matmuls start while the second half still streams
            nc.sync.dma_start(out=wqk[:, 0:4, 0:256], in_=wqk_r[:, 0:4, 0:256])
            nc.sync.dma_start(out=xt[:, 0:4, 0:512], in_=xT_r[:, 0:4, 0:512])
            nc.sync.dma_start(out=wqk[:, 4:8, 0:256], in_=wqk_r[:, 4:8, 0:256])
            nc.sync.dma_start(out=xt[:, 4:8, 0:512], in_=xT_r[:, 4:8, 0:512])
            for nb in range(1, NB):
                nc.sync.dma_start(out=xt[:, :, nb * 512:(nb + 1) * 512],
                                  in_=xT_r[:, :, nb * 512:(nb + 1) * 512])
            nc.sync.dma_start(out=wv, in_=wv_r)
            for pr in range(1, 4):
                nc.sync.dma_start(out=wqk[:, :, pr * 256:(pr + 1) * 256],
                                  in_=wqk_r[:, :, pr * 256:(pr + 1) * 256])
            nc.sync.dma_start(out=wp, in_=wp_r)

            ones = big.tile([1, 64], bf16, tag="ones")
            nc.vector.memset(ones, 1.0)

            # ---- persistent intermediates ----
            qkT = [big.tile([128, N], bf16, tag=f"qkT{i}", name=f"qkT{i}") for i in range(QKT)]
            v1 = [big.tile([128, HPC * 65], bf16, tag=f"v1_{i}", name=f"v1_{i}") for i in range(NT)]
            attnT = [big.tile([128, N], bf16, tag=f"attnT{i}", name=f"attnT{i}") for i in range(4)]

            def qk_group(mt, nb):
                # wqk is pair-major: q cols of pair p at p*256, k at p*256+128
                co = (mt % 4) * 256 + (mt // 4) * 128
                ps = mmp.tile([128, 512], f32, tag="mm")
                for kt in range(KT):
                    nc.tensor.matmul(
                        ps,
                        lhsT=wqk[:, kt, co:co + 128],
                        rhs=xt[:, kt, nb * 512:(nb + 1) * 512],
                        start=(kt == 0),
                        stop=(kt == KT - 1),
                    )
                nc.vector.tensor_copy(
                    out=qkT[mt][:, nb * 512:(nb + 1) * 512], in_=ps
                )

            def qk_tile(mt):
                """qkT[mt] = (wqk[:, mt*128:+128]).T @ xT  -> [128, 2048]"""
                for nb in range(NB):
                    qk_group(mt, nb)

            def v_tile(nt):
                """v1[nt][:, h*65:h*65+64] = V rows nt*128..; col h*65+64 = 1"""
                ps = mmp.tile([128, 512], f32, tag="mm")
                for kt in range(KT):
                    nc.tensor.matmul(
                        ps,
                        lhsT=xt[:, kt, nt * 128:(nt + 1) * 128],
                        rhs=wv[:, kt, :],
                        start=(kt == 0),
                        stop=(kt == KT - 1),
                    )
                v3 = v1[nt].rearrange("p (h e) -> p h e", e=65)
                nc.vector.memset(v3[:, :, 64:65], 1.0)
                nc.vector.tensor_copy(
                    out=v3[:, :, 0:64],
                    in_=ps.rearrange("p (h e) -> p h e", e=64),
                )

            rb_idx = [0]

            def s_units(h, ng, fillers, dense=False):
                """Generator of 16 S-phase units for block (h, ng): each emits
                the two S matmuls + the exp, plus an optional filler group.
                Appends e tiles to the returned list as units run.  With
                `dense`, one filler is popped at every mt (slot i == mt i, for
                fillers that must land before a specific S/PV consumer)."""
                fillers = list(fillers)
                if dense:
                    slots = set(range(NT))
                elif fillers:
                    stride = max(1, NT // len(fillers))
                    slots = set(range(stride - 1, NT, stride))
                else:
                    slots = set()
                qt = qkT[h // 2]
                kt_ = qkT[4 + h // 2]
                po = (h % 2) * 64
                es = []

                def gen():
                    for mt in range(NT):
                        sp = spsum.tile([128, 1024], f32, tag="sps", name="sp")
                        for half in range(2):
                            nc.tensor.matmul(
                                sp[:, half * 512:(half + 1) * 512],
                                lhsT=kt_[po:po + 64, mt * 128:(mt + 1) * 128],
                                rhs=qt[po:po + 64,
                                       ng * 1024 + half * 512:
                                       ng * 1024 + (half + 1) * 512],
                                start=True,
                                stop=True,
                            )
                        e = workp.tile([128, 1024], bf16, tag="e", name="e")
                        nc.scalar.activation(
                            out=e, in_=sp,
                            func=mybir.ActivationFunctionType.Exp,
                            scale=0.125,
                        )
                        es.append(e)
                        if fillers and mt in slots:
                            fillers.pop(0)()
                        yield
                    while fillers:  # leftovers
                        fillers.pop(0)()

                return es, gen()

            def pv_units(h, ng, es, fillers=(), pe_norm=False):
                """Generator of 32 PV matmul units for block (h, ng); after
                exhaustion emits the two normalization chains.  `fillers` are
                popped one per mt (used to finish V tiles ahead of their PV
                use).  `pe_norm` broadcasts 1/denom with a PE matmul instead
                of the DRAM bounce (shorter latency; used for late blocks on
                the critical path to proj)."""
                fillers = list(fillers)
                po = (h % 2) * 64
                at = attnT[h // 2]
                pvs = [pvpsum.tile([65, 512], f32, tag="pv", name="pv")
                       for _ in range(2)]

                def norm(half):
                    pv = pvs[half]
                    r = smallp.tile([1, 512], bf16, tag="r", name="r")
                    with nc.allow_low_precision(reason="softmax recip bcast"):
                        nc.vector.reciprocal(out=r, in_=pv[64:65, :])
                    # copy the numerator out of PSUM immediately so the pv
                    # slot frees for the next block's PV without waiting for
                    # the broadcast round-trip
                    pvsb = smallp.tile([64, 512], f32, tag="pvsb", name="pvsb")
                    nc.vector.tensor_copy(out=pvsb, in_=pv[0:64, :])
                    rbs = smallp.tile([64, 512], f32, tag="rbs", name="rbs")
                    if pe_norm:
                        rb = mmp.tile([64, 512], f32, tag="mm", name="rb")
                        nc.tensor.matmul(rb, lhsT=ones, rhs=r,
                                         start=True, stop=True)
                        nc.vector.tensor_copy(out=rbs, in_=rb)
                    else:
                        idx = rb_idx[0] % 32
                        rb_idx[0] += 1
                        nc.sync.dma_start(out=rdram[idx], in_=r[0, :])
                        rsl = rdram[idx]
                        bcast = bass.AP(tensor=rsl.tensor, offset=rsl.offset,
                                        ap=[[0, 64]] + [list(p) for p in rsl.ap])
                        nc.sync.dma_start(out=rbs, in_=bcast)
                    nc.vector.tensor_mul(
                        out=at[po:po + 64,
                               ng * 1024 + half * 512:
                               ng * 1024 + (half + 1) * 512],
                        in0=pvsb,
                        in1=rbs,
                    )

                def gen():
                    for mt in range(NT):
                        if fillers:
                            fillers.pop(0)()
                        for half in range(2):
                            nc.tensor.matmul(
                                pvs[half],
                                lhsT=v1[mt][:, h * 65:(h + 1) * 65],
                                rhs=es[mt][:, half * 512:(half + 1) * 512],
                                start=(mt == 0),
                                stop=(mt == NT - 1),
                            )
                            yield
                    norm(0)
                    norm(1)

                return gen()

            def run_all(g):
                for _ in g:
                    pass

            def interleave(sgen, pvgen):
                """2 PV units per S unit (32 PV vs 16 S per block)."""
                while True:
                    done = 0
                    for _ in range(2):
                        if next(pvgen, StopIteration) is StopIteration:
                            done += 1
                            break
                    if next(sgen, StopIteration) is StopIteration:
                        done += 1
                    if done:
                        for _ in pvgen:
                            pass
                        for _ in sgen:
                            pass
                        return

            def proj(nt):
                # the second half of proj runs after the last attention block:
                # rotate over the then-idle pv/sps PSUM slots too, so groups
                # aren't serialized on the two "mm" slots
                if nt < 8:
                    pool_tag = (mmp, "mm")
                else:
                    pool_tag = [(mmp, "mm"), (pvpsum, "pv"), (spsum, "sps")][nt % 3]
                ot = outp.tile([128, C], f32, tag="ot")
                for ob in range(2):
                    ps = pool_tag[0].tile([128, 512], f32, tag=pool_tag[1])
                    for ct in range(QKC // 128):
                        nc.tensor.matmul(
                            ps,
                            lhsT=attnT[ct][:, nt * 128:(nt + 1) * 128],
                            rhs=wp[:, ct, ob * 512:(ob + 1) * 512],
                            start=(ct == 0),
                            stop=(ct == QKC // 128 - 1),
                        )
                    # ScalarE is idle during the projection tail; DVE is not
                    nc.scalar.copy(
                        out=ot[:, ob * 512:(ob + 1) * 512], in_=ps
                    )
                nc.sync.dma_start(
                    out=out_d[nt * 128:(nt + 1) * 128, :], in_=ot
                )

            # Software pipeline over 16 (h, ng) blocks: block i's S-phase (the
            # exp feed) interleaves with block i-1's PV matmuls so ScalarE
            # never starves at head boundaries.  Only qk tiles 0 and 4 precede
            # attention; V tiles are built as fillers inside blocks 0/1, later
            # qk tile-groups inside earlier pairs' blocks (always complete
            # before first use).  The last block's PV overlaps the output
            # projection, and the last two blocks normalize via PE broadcast
            # (short latency) instead of the DRAM bounce.
            import functools
            for _rep in range(repeat):
              # Minimal prelude: S(b0=(h0,ng0), mt) needs q cols 0:1024
              # (qk groups (0,0),(0,1)) and k block nb0 (group (4,0)); the
              # other qk(0)/qk(4) groups and V tiles ride as dense fillers
              # inside block 0, ordered so each lands before its first
              # consumer (group (4,j) before S mt=4j; v1[i] before PV mt i).
              qk_group(0, 0)
              qk_group(0, 1)
              qk_group(4, 0)

              qkg = [[functools.partial(qk_group, m, nb) for nb in range(NB)]
                   for m in range(QKT)]
              vg = [functools.partial(v_tile, nt) for nt in range(NT)]
              block_fill = [[] for _ in range(16)]
              block_fill[0] = [
                  qkg[4][1], qkg[0][2], qkg[0][3], vg[0],
                  qkg[4][2], vg[1], vg[2], vg[3],
                  qkg[4][3], vg[4], vg[5], vg[6],
                  vg[7], vg[8], vg[9], vg[10],
              ]
              pv0_fill = vg[11:16]       # v1[11..15] paced inside PV(b0)
              f15 = qkg[1] + qkg[5]      # tiles 1,5 for head pair 1 (blocks 4-7)
              block_fill[2] = f15[0:4]
              block_fill[3] = f15[4:8]
              f26 = qkg[2] + qkg[6]      # tiles 2,6 for pair 2 (blocks 8-11)
              for i in range(4):
                  block_fill[4 + i] = f26[2 * i:2 * i + 2]
              f37 = qkg[3] + qkg[7]      # tiles 3,7 for pair 3 (blocks 12-15)
              for i in range(4):
                  block_fill[8 + i] = f37[2 * i:2 * i + 2]

              blocks = [(h, ng) for h in range(HPC) for ng in range(2)]
              prev_pv = None
              for bi, (h, ng) in enumerate(blocks):
                  es, sgen = s_units(h, ng, block_fill[bi], dense=(bi == 0))
                  if prev_pv is None:
                      run_all(sgen)
                  else:
                      interleave(sgen, prev_pv)
                  prev_pv = pv_units(
                      h, ng, es,
                      fillers=pv0_fill if bi == 0 else (),
                      pe_norm=True,
                  )

              # tail: last block's PV interleaved with the first half of the
              # projection (those rows need only norm(14), already done); then
              # its norm (PE broadcast, short), then the remaining projection.
              for nt in range(8):
                  for _ in range(4):
                      next(prev_pv, None)
                  proj(nt)
              run_all(prev_pv)
              for nt in range(8, NT):
                  proj(nt)

    if split_waits:
        _split_multi_waits(nc, mybir)
    return nc


def _split_multi_waits(nc, mybir):
    """TPB instructions carry exactly one sync-wait slot; walrus codegen
    rejects instructions Tile scheduled with >1 waits ("Too many sync wait
    commands").  Hoist all but the last wait onto NoOps inserted just before
    the instruction on the same engine queue (queues execute in order, so
    semantics are identical)."""
    eng_ok = {
        mybir.EngineType.PE,
        mybir.EngineType.Activation,
        mybir.EngineType.DVE,
        mybir.EngineType.Pool,
        mybir.EngineType.SP,
    }
    k = 0
    for f in nc.m.functions:
        for blk in f.blocks:
            out = []
            changed = False
            for inst in blk.instructions:
                si = inst.sync_info
                if (
                    si is not None
                    and len(si.on_wait) > 1
                    and inst.engine in eng_ok
                ):
                    waits = list(si.on_wait)
                    for w in waits[:-1]:
                        nop = mybir.InstNoOp(name=f"I-splitw-{k}", ins=[], outs=[])
                        k += 1
                        nop.engine = inst.engine
                        nop.sync_info = mybir.SyncInfo(on_wait=[w], on_update=[])
                        out.append(nop)
                    inst.sync_info = mybir.SyncInfo(
                        on_wait=[waits[-1]], on_update=list(si.on_update)
                    )
                    changed = True
                out.append(inst)
            if changed:
                blk.instructions = out


def _get_nc():
    global _nc_cache
    if _nc_cache is None:
        _nc_cache = build_nc()
    return _nc_cache


def make_in_maps(x, W_qkv, W_proj):
    bf16 = ml_dtypes.bfloat16
    in_maps = []
    for c in range(NCORES):
        b, g = divmod(c, 2)
        xT = np.ascontiguousarray(np.asarray(x[b]).T).astype(bf16)
        wq = W_qkv[:, g * QKC:(g + 1) * QKC]
        wk = W_qkv[:, C + g * QKC:C + (g + 1) * QKC]
        # pair-major: [q128 | k128] per head pair, matching qk_group's co map
        wqk = np.concatenate(
            [blk for p in range(4)
             for blk in (wq[:, p * 128:(p + 1) * 128],
                         wk[:, p * 128:(p + 1) * 128])],
            axis=1,
        ).astype(bf16)
        wv = np.ascontiguousarray(W_qkv[:, 2 * C + g * QKC:2 * C + (g + 1) * QKC]).astype(bf16)
        wp = np.ascontiguousarray(W_proj[g * QKC:(g + 1) * QKC, :]).astype(bf16)
        in_maps.append({"xT": xT, "wqk": wqk, "wv": wv, "wp": wp})
    return in_maps


last_exec_time_ns = None


def kernel(x, W_qkv, W_proj, b_proj):
    global last_exec_time_ns
    import os
    # the NTFF trace path needs antenv.axon_hooks, absent in this container
    os.environ["BASS_NEVER_TRACE"] = "1"
    from concourse import bass_utils

    x = np.asarray(x)
    W_qkv = np.asarray(W_qkv)
    W_proj = np.asarray(W_proj)
    b_proj = np.asarray(b_proj)

    nc = _get_nc()
    in_maps = make_in_maps(x, W_qkv, W_proj)
    res = bass_utils.run_bass_kernel_spmd(nc, in_maps, list(range(NCORES)))
    last_exec_time_ns = res.exec_time_ns

    out = np.empty((B, N, C), np.float32)
    bias = b_proj.astype(np.float32)
    for b in range(B):
        out[b] = res.results[2 * b]["out"] + res.results[2 * b + 1]["out"] + bias
    return out



# revision 2
# speedup vs baseline: 985.0892x; 985.0892x over previous
"""Multi-head attention (B=4, N=2048, C=1024, H=16, D=64) on 8 trn2 cores.

v2: fp8 DoubleRow attention core.
  - QKV projection in bf16 (error budget forbids fp8 here).
  - q,k stored fp8 in DoubleRow layout [32 parts, 2 d-halves, N] via
    psum->fp8 copies + SBUF->SBUF partition-shift DMAs.
  - S^T = k^T q as ONE DoubleRow matmul per (mt, 512-col half): contraction
    d=64 = 32 partitions x 2 pair-planes; cost 0.5 cyc/col.
  - exp split across ScalarE (native) and DVE/Pool (Schraudolph bit-trick:
    e = bitcast_f32(int32(s*2^23/ln2*0.125 + B))), output fp8 pairs
    e2[c] = [128, 2, 1024] (planes = mt 2c, 2c+1).
  - PV as DoubleRow over m-tile pairs: lhsT [128, 2, 65] = [V|ones] pairs,
    plus a residual chain lhsT [128, 2, 64] = fp8(V - fp8(V)) accumulated
    into the same psum rows 0:64 (corrects the V fp8 storage error).
  - norm via PE broadcast of 1/denom; projection in bf16.

Sharding: core c = (batch b = c//2, head-half g = c%2), host sums partials.
"""

import numpy as np
import ml_dtypes

B, N, C = 4, 2048, 1024
H, D = 16, 64
HPC = 8            # heads per core
QKC = HPC * D      # 512 q (and k, v) channels per core
NCORES = 8

# exp engine assignment per mt (16 per block): 's'=ScalarE exact exp,
# 'p'=Schraudolph (step1 tensor_scalar on DVE - GPSIMD can't read PSUM -
# then bitcast->fp8 copy on Pool), 'd'=Schraudolph fully on DVE
EXP_ASSIGN = ['s', 'p', 's', 's', 's', 'p', 's', 's',
              's', 's', 'p', 's', 's', 's', 's', 's']
# Schraudolph constants (floor semantics), scale 0.125 folded in
SCH_A = float(2 ** 23 / np.log(2) * 0.125)
SCH_B = float(127 * 2 ** 23 - 486411.0)
# reciprocal bit-trick seed: r0 = bitcast(MAGIC - bits(d)); one Newton
# step brings 5.1% seed error to 0.26% on the softmax denominator range
RCP_MAGIC = float(0x7EF30000)

_nc_cache = None


def build_nc(split_waits=True, repeat=1):
    import concourse.bass as bass
    import concourse.mybir as mybir
    import concourse.tile as tile

    bf16 = mybir.dt.bfloat16
    f32 = mybir.dt.float32
    f8 = mybir.dt.float8e4
    i32 = mybir.dt.int32
    DRm = mybir.MatmulPerfMode.DoubleRow
    Alu = mybir.AluOpType

    nc = bass.Bass()
    xT_d = nc.declare_dram_parameter("xT", [C, N], bf16, isOutput=False)
    wqk_d = nc.declare_dram_parameter("wqk", [C, 2 * QKC], bf16, isOutput=False)
    wv_d = nc.declare_dram_parameter("wv", [C, QKC], bf16, isOutput=False)
    wp_d = nc.declare_dram_parameter("wp", [QKC, C], bf16, isOutput=False)
    out_d = nc.declare_dram_parameter("out", [N, C], f32, isOutput=True)

    KT = C // 128          # 8 contraction tiles for qkv projection
    NT = N // 128          # 16 seq tiles
    NB = N // 512          # 4 seq blocks of 512
    QKT = 2 * QKC // 128   # 8 qk-channel tiles

    with tile.TileContext(nc) as tc:
        with (
            tc.tile_pool(name="big", bufs=1) as big,
            tc.tile_pool(name="e2p", bufs=14) as e2p,
            tc.tile_pool(name="qk8p", bufs=3) as qk8p,
            tc.tile_pool(name="outp", bufs=4) as outp,
            tc.tile_pool(name="small", bufs=2) as smallp,
            tc.tile_pool(name="schp", bufs=4) as schp,
            tc.tile_pool(name="mm", bufs=2, space="PSUM") as mmp,
            tc.tile_pool(name="spsum", bufs=2, space="PSUM") as spsum,
            tc.tile_pool(name="pvpsum", bufs=2, space="PSUM") as pvpsum,
        ):
            # ---- load inputs ----
            xt = big.tile([128, KT, N], bf16, tag="xt")
            wqk = big.tile([128, KT, 2 * QKC], bf16, tag="wqk")
            wv = big.tile([128, KT, QKC], bf16, tag="wv")
            wp = big.tile([128, QKC // 128, C], bf16, tag="wp")
            xT_r = xT_d.rearrange("(t p) n -> p t n", p=128)
            wqk_r = wqk_d.rearrange("(t p) m -> p t m", p=128)
            wv_r = wv_d.rearrange("(t p) m -> p t m", p=128)
            wp_r = wp_d.rearrange("(t p) m -> p t m", p=128)
            nc.sync.dma_start(out=wqk[:, 0:4, 0:256], in_=wqk_r[:, 0:4, 0:256])
            nc.sync.dma_start(out=xt[:, 0:4, 0:512], in_=xT_r[:, 0:4, 0:512])
            nc.sync.dma_start(out=wqk[:, 4:8, 0:256], in_=wqk_r[:, 4:8, 0:256])
            nc.sync.dma_start(out=xt[:, 4:8, 0:512], in_=xT_r[:, 4:8, 0:512])
            for nb in range(1, NB):
                nc.sync.dma_start(out=xt[:, :, nb * 512:(nb + 1) * 512],
                                  in_=xT_r[:, :, nb * 512:(nb + 1) * 512])
            nc.sync.dma_start(out=wv, in_=wv_r)
            for pr in range(1, 4):
                nc.sync.dma_start(out=wqk[:, :, pr * 256:(pr + 1) * 256],
                                  in_=wqk_r[:, :, pr * 256:(pr + 1) * 256])
            nc.sync.dma_start(out=wp, in_=wp_r)

            ones = big.tile([1, 64], bf16, tag="ones")
            nc.vector.memset(ones, 1.0)

            # ---- persistent intermediates ----
            # q/k fp8 DoubleRow tiles: QD[t] holds heads 4t..4t+3, head h at
            # partitions 32*(h%4); plane i = d-half i (d = 32*i + p).
            QD = [big.tile([128, 2, N], f8, tag=f"qd{t}", name=f"qd{t}")
                  for t in range(2)]
            KD = [big.tile([128, 2, N], f8, tag=f"kd{t}", name=f"kd{t}")
                  for t in range(2)]
            # V fp8 pair tiles: plane i = m-tile 2c+i; per head 65 cols
            # (64 V + ones); VR = fp8 residual (64 cols, no ones).
            # 66-wide planes: [V(64) | ones | zero-pad] - dual-fp8 ldweights
            # rejects odd stationary plane widths (s3_lw_dual_fp8_restrictions)
            V8 = [big.tile([128, 2, HPC * 66], f8, tag=f"v8_{c}", name=f"v8_{c}")
                  for c in range(NT // 2)]
            VR = [big.tile([128, 2, HPC * 64], f8, tag=f"vr_{c}", name=f"vr_{c}")
                  for c in range(NT // 2)]
            attnT = [big.tile([128, N], bf16, tag=f"attnT{i}", name=f"attnT{i}")
                     for i in range(4)]

            def qk_group(mt, nb, copy_eng):
                """bf16 QKV matmul group -> fp8 staging copy.
                Returns psum handle; copy lands in qk8 staging tile."""
                co = (mt % 4) * 256 + (mt // 4) * 128
                ps = mmp.tile([128, 512], f32, tag="mm")
                for kt in range(KT):
                    nc.tensor.matmul(
                        ps,
                        lhsT=wqk[:, kt, co:co + 128],
                        rhs=xt[:, kt, nb * 512:(nb + 1) * 512],
                        start=(kt == 0),
                        stop=(kt == KT - 1),
                    )
                nc.vector.tensor_copy(
                    out=qk8_stage[mt][:, nb * 512:(nb + 1) * 512], in_=ps)

            def qk_shuffle(mt):
                """4 partition-shift DMAs: qk8_stage[mt] [128,2048] fp8 ->
                DR layout. Stage tile holds heads 2p, 2p+1 (p = mt%4);
                q if mt<4 else k."""
                p = mt % 4
                dst = QD if mt < 4 else KD
                for hh in range(2):        # head within stage tile
                    h = 2 * p + hh
                    for i in range(2):     # d-half plane
                        nc.sync.dma_start(
                            out=dst[h // 4][32 * (h % 4):32 * (h % 4) + 32, i, :],
                            in_=qk8_stage[mt][64 * hh + 32 * i:
                                              64 * hh + 32 * i + 32, :],
                        )

            def v_tile(nt, copy_eng):
                """v8/vres pair-plane build for m-tile nt."""
                ps = mmp.tile([128, 512], f32, tag="mm")
                for kt in range(KT):
                    nc.tensor.matmul(
                        ps,
                        lhsT=xt[:, kt, nt * 128:(nt + 1) * 128],
                        rhs=wv[:, kt, :],
                        start=(kt == 0),
                        stop=(kt == KT - 1),
                    )
                c, i = nt // 2, nt % 2
                v3 = V8[c][:, i, :].rearrange("p (h e) -> p h e", e=66)
                r3 = VR[c][:, i, :].rearrange("p (h e) -> p h e", e=64)
                nc.gpsimd.memset(v3[:, :, 64:65], 1.0)
                nc.gpsimd.memset(v3[:, :, 65:66], 0.0)
                nc.vector.tensor_copy(
                    out=v3[:, :, 0:64],
                    in_=ps.rearrange("p (h e) -> p h e", e=64),
                )
                # residual: fp8(psum - fp8(v))
                nc.vector.tensor_tensor(
                    out=r3,
                    in0=ps.rearrange("p (h e) -> p h e", e=64),
                    in1=v3[:, :, 0:64],
                    op=mybir.AluOpType.subtract,
                )

            def s_units(h, ng, fillers, dense=False):
                """16 S-phase units: one DoubleRow matmul pair + exp."""
                fillers = list(fillers)
                if dense:
                    slots = set(range(NT))
                elif fillers:
                    stride = max(1, NT // len(fillers))
                    slots = set(range(stride - 1, NT, stride))
                else:
                    slots = set()
                t, pb = h // 4, 32 * (h % 4)
                es = []  # (e2_tile, plane) per mt

                def gen():
                    for mt in range(NT):
                        sp = spsum.tile([128, 1024], f32, tag="sps", name="sp")
                        for half in range(2):
                            nc.tensor.matmul(
                                sp[:, half * 512:(half + 1) * 512],
                                lhsT=KD[t][pb:pb + 32, :, mt * 128:(mt + 1) * 128],
                                rhs=QD[t][pb:pb + 32, :,
                                          ng * 1024 + half * 512:
                                          ng * 1024 + (half + 1) * 512],
                                start=True,
                                stop=True,
                                perf_mode=DRm,
                                tile_position=(pb, 0),
                            )
                        c, i = mt // 2, mt % 2
                        if i == 0:
                            e2 = e2p.tile([128, 2, 1024], f8, tag="e2", name="e2")
                            es.append((e2, 0))
                        else:
                            e2 = es[-1][0]
                            es.append((e2, 1))
                        eng = EXP_ASSIGN[mt]
                        if eng == 's':
                            nc.scalar.activation(
                                out=e2[:, i, :], in_=sp,
                                func=mybir.ActivationFunctionType.Exp,
                                scale=0.125,
                            )
                        else:
                            tmp = schp.tile([128, 1024], i32, tag="sch",
                                            name="sch")
                            nc.vector.tensor_scalar(
                                out=tmp, in0=sp,
                                scalar1=SCH_A, scalar2=SCH_B,
                                op0=Alu.mult, op1=Alu.add,
                            )
                            e2nd = nc.gpsimd if eng == 'p' else nc.vector
                            e2nd.tensor_copy(out=e2[:, i, :],
                                             in_=tmp.bitcast(f32))
                        if fillers and mt in slots:
                            fillers.pop(0)()
                        yield
                    while fillers:
                        fillers.pop(0)()

                return es, gen()

            def pv_units(h, ng, es, fillers=()):
                """16 PV units (pair c, half): DoubleRow main [V|ones] + res
                chain into psum rows 0:64; then the two normalizations."""
                fillers = list(fillers)
                po = (h % 2) * 64
                at = attnT[h // 2]
                pvs = [pvpsum.tile([66, 512], f32, tag="pv", name="pv")
                       for _ in range(2)]

                def norm(half):
                    # 1/denom without InstReciprocal (microcoded, 4us) or
                    # custom-DVE ISA (walrus rejects): magic-seed + one
                    # Newton step, all on Pool (SBUF-only engine); DVE only
                    # touches psum (denom copy-out, final multiply).
                    pv = pvs[half]
                    d32 = smallp.tile([1, 512], f32, tag="d32", name="d32")
                    nc.vector.tensor_copy(out=d32, in_=pv[64:65, :])
                    r0i = smallp.tile([1, 512], i32, tag="r0i", name="r0i")
                    nc.gpsimd.tensor_scalar(
                        out=r0i, in0=d32.bitcast(i32),
                        scalar1=-1.0, scalar2=RCP_MAGIC,
                        op0=Alu.mult, op1=Alu.add)
                    r0 = r0i.bitcast(f32)
                    t1 = smallp.tile([1, 512], f32, tag="t1", name="t1")
                    nc.gpsimd.tensor_tensor(out=t1, in0=d32, in1=r0,
                                            op=Alu.mult)
                    t2 = smallp.tile([1, 512], f32, tag="t2", name="t2")
                    nc.gpsimd.tensor_scalar(
                        out=t2, in0=t1, scalar1=-1.0, scalar2=2.0,
                        op0=Alu.mult, op1=Alu.add)
                    r = smallp.tile([1, 512], bf16, tag="r", name="r")
                    with nc.allow_low_precision(reason="softmax recip bcast"):
                        nc.gpsimd.tensor_tensor(out=r, in0=t2, in1=r0,
                                                op=Alu.mult)
                    rb = mmp.tile([64, 512], f32, tag="mm", name="rb")
                    nc.tensor.matmul(rb, lhsT=ones, rhs=r, start=True, stop=True)
                    rbs = smallp.tile([64, 512], f32, tag="rbs", name="rbs")
                    nc.vector.tensor_copy(out=rbs, in_=rb)
                    nc.vector.tensor_mul(
                        out=at[po:po + 64,
                               ng * 1024 + half * 512:
                               ng * 1024 + (half + 1) * 512],
                        in0=pv[0:64, :],
                        in1=rbs,
                    )

                def gen():
                    n_c = NT // 2
                    for c in range(n_c):
                        if fillers:
                            fillers.pop(0)()
                        for half in range(2):
                            rhs = es[2 * c][0][:, :, half * 512:(half + 1) * 512]
                            nc.tensor.matmul(
                                pvs[half],
                                lhsT=V8[c][:, :, h * 66:(h + 1) * 66],
                                rhs=rhs,
                                start=(c == 0),
                                stop=False,
                                perf_mode=DRm,
                                skip_group_check=True,
                            )
                            nc.tensor.matmul(
                                pvs[half][0:64, :],
                                lhsT=VR[c][:, :, h * 64:(h + 1) * 64],
                                rhs=rhs,
                                start=False,
                                stop=(c == n_c - 1),
                                perf_mode=DRm,
                                skip_group_check=True,
                            )
                            yield
                    norm(0)
                    norm(1)

                return gen()

            def run_all(g):
                for _ in g:
                    pass

            def interleave(sgen, pvgen):
                """1 PV unit per S unit (16 each per block)."""
                while True:
                    done = 0
                    if next(pvgen, StopIteration) is StopIteration:
                        done += 1
                    if next(sgen, StopIteration) is StopIteration:
                        done += 1
                    if done:
                        for _ in pvgen:
                            pass
                        for _ in sgen:
                            pass
                        return

            def proj(nt):
                if nt < 8:
                    pool_tag = (mmp, "mm")
                else:
                    pool_tag = [(mmp, "mm"), (pvpsum, "pv"), (spsum, "sps")][nt % 3]
                ot = outp.tile([128, C], f32, tag="ot")
                for ob in range(2):
                    ps = pool_tag[0].tile([128, 512], f32, tag=pool_tag[1])
                    for ct in range(QKC // 128):
                        nc.tensor.matmul(
                            ps,
                            lhsT=attnT[ct][:, nt * 128:(nt + 1) * 128],
                            rhs=wp[:, ct, ob * 512:(ob + 1) * 512],
                            start=(ct == 0),
                            stop=(ct == QKC // 128 - 1),
                        )
                    nc.scalar.copy(
                        out=ot[:, ob * 512:(ob + 1) * 512], in_=ps
                    )
                nc.sync.dma_start(
                    out=out_d[nt * 128:(nt + 1) * 128, :], in_=ot
                )

            import functools
            for _rep in range(repeat):
                # fresh staging tiles each repeat (pool rotates)
                qk8_stage = [qk8p.tile([128, N], f8, tag="qk8",
                                       name=f"qk8_{_rep}_{m}")
                             for m in range(QKT)]

                # Prelude: q tile 0 + k tile 4 complete + shuffled, so
                # S(h0) can start. Alternate copy engines.
                for nb in range(NB):
                    qk_group(0, nb, 'd' if nb % 2 else 'p')
                qk_shuffle(0)
                for nb in range(NB):
                    qk_group(4, nb, 'd' if nb % 2 else 'p')
                qk_shuffle(4)

                def g_and_s(mt):
                    def f(nb, mt=mt):
                        qk_group(mt, nb, 'd' if nb % 2 else 'p')
                        if nb == NB - 1:
                            qk_shuffle(mt)
                    return [functools.partial(f, nb) for nb in range(NB)]

                vg = [functools.partial(v_tile, nt, 'd' if nt % 2 else 'p')
                      for nt in range(NT)]
                block_fill = [[] for _ in range(16)]
                # tile 1 (heads 2,3) + k tile 5 inside blocks 0-1;
                # v tiles as dense fillers in block 0 + pv0 fillers.
                f15 = g_and_s(1) + g_and_s(5)
                block_fill[0] = ([f15[0], f15[1], vg[0], vg[1],
                                  f15[2], f15[3], vg[2], vg[3],
                                  f15[4], f15[5], vg[4], vg[5],
                                  f15[6], f15[7], vg[6], vg[7]])
                pv0_fill = vg[8:16]
                f26 = g_and_s(2) + g_and_s(6)   # heads 4,5 for blocks 8-11
                for i in range(4):
                    block_fill[2 + i] = f26[2 * i:2 * i + 2]
                f37 = g_and_s(3) + g_and_s(7)   # heads 6,7 for blocks 12-15
                for i in range(4):
                    block_fill[6 + i] = f37[2 * i:2 * i + 2]

                blocks = [(h, ng) for h in range(HPC) for ng in range(2)]
                prev_pv = None
                for bi, (h, ng) in enumerate(blocks):
                    es, sgen = s_units(h, ng, block_fill[bi], dense=(bi == 0))
                    if prev_pv is None:
                        run_all(sgen)
                    else:
                        interleave(sgen, prev_pv)
                    prev_pv = pv_units(
                        h, ng, es,
                        fillers=pv0_fill if bi == 0 else (),
                    )

                # tail: last block's PV interleaved with first half of proj
                for nt in range(8):
                    for _ in range(2):
                        next(prev_pv, None)
                    proj(nt)
                run_all(prev_pv)
                for nt in range(8, NT):
                    proj(nt)

    if split_waits:
        _split_multi_waits(nc, mybir)
    return nc


def _split_multi_waits(nc, mybir):
    """TPB instructions carry exactly one sync-wait slot; hoist extra waits
    onto same-queue NoOps."""
    eng_ok = {
        mybir.EngineType.PE,
        mybir.EngineType.Activation,
        mybir.EngineType.DVE,
        mybir.EngineType.Pool,
        mybir.EngineType.SP,
    }
    k = 0
    for f in nc.m.functions:
        for blk in f.blocks:
            out = []
            changed = False
            for inst in blk.instructions:
                si = inst.sync_info
                if (
                    si is not None
                    and len(si.on_wait) > 1
                    and inst.engine in eng_ok
                ):
                    waits = list(si.on_wait)
                    for w in waits[:-1]:
                        nop = mybir.InstNoOp(name=f"I-splitw-{k}", ins=[], outs=[])
                        k += 1
                        nop.engine = inst.engine
                        nop.sync_info = mybir.SyncInfo(on_wait=[w], on_update=[])
                        out.append(nop)
                    inst.sync_info = mybir.SyncInfo(
                        on_wait=[waits[-1]], on_update=list(si.on_update)
                    )
                    changed = True
                out.append(inst)
            if changed:
                blk.instructions = out
    return nc


def _get_nc():
    global _nc_cache
    if _nc_cache is None:
        _nc_cache = build_nc()
    return _nc_cache


def make_in_maps(x, W_qkv, W_proj):
    bf16 = ml_dtypes.bfloat16
    in_maps = []
    for c in range(NCORES):
        b, g = divmod(c, 2)
        xT = np.ascontiguousarray(np.asarray(x[b]).T).astype(bf16)
        wq = W_qkv[:, g * QKC:(g + 1) * QKC]
        wk = W_qkv[:, C + g * QKC:C + (g + 1) * QKC]
        wqk = np.concatenate(
            [blk for p in range(4)
             for blk in (wq[:, p * 128:(p + 1) * 128],
                         wk[:, p * 128:(p + 1) * 128])],
            axis=1,
        ).astype(bf16)
        wv = np.ascontiguousarray(
            W_qkv[:, 2 * C + g * QKC:2 * C + (g + 1) * QKC]).astype(bf16)
        wp = np.ascontiguousarray(W_proj[g * QKC:(g + 1) * QKC, :]).astype(bf16)
        in_maps.append({"xT": xT, "wqk": wqk, "wv": wv, "wp": wp})
    return in_maps


last_exec_time_ns = None


def kernel(x, W_qkv, W_proj, b_proj):
    global last_exec_time_ns
    import os
    os.environ["BASS_NEVER_TRACE"] = "1"
    from concourse import bass_utils

    x = np.asarray(x)
    W_qkv = np.asarray(W_qkv)
    W_proj = np.asarray(W_proj)
    b_proj = np.asarray(b_proj)

    nc = _get_nc()
    in_maps = make_in_maps(x, W_qkv, W_proj)
    res = bass_utils.run_bass_kernel_spmd(nc, in_maps, list(range(NCORES)))
    last_exec_time_ns = res.exec_time_ns

    out = np.empty((B, N, C), np.float32)
    bias = b_proj.astype(np.float32)
    for b in range(B):
        out[b] = res.results[2 * b]["out"] + res.results[2 * b + 1]["out"] + bias
    return out
